# revision 42
# baseline (speedup 1.0000x reference)
"""Trainium2 Bass kernel for nn_AutoCorrelation_spa_tem.

Shards batch B=32 across 8 NeuronCores (4 batches/core, pure data parallel).

Algorithm (collapsed form of the reference):
  G_b   = keys[b](L,HE) @ queries[b](L,HE)^T            (192x192)
  D_raw[b,tau] = sum_s G_b[s,(s+tau)%L]                 (diag sums via shear)
  gsum  = AllGather_b(sum_b D_raw) + local sum -> top-5 mask via max8
  c_b   = mask * softmax(D_raw[b]/HE over selected)
  W_b   = keys[b].reshape(HE,L)^T @ values_proper(HE,L) (192x192)
  M_b   = sum_d c_b[d] * Shift_d(W_b)   [2D circular diagonal shift]
        = unshear(HankelC^T @ shear(W_b))   (all positive-stride DMAs)
  out[b] = (Qtilde_b @ M_b)^T  computed TRANSPOSED: outT = qr^T-chunks @ mrev
           (host transposes back; 192-col matmuls instead of 512-col)

v2 perf structure (vs v1 ~113-120us):
  - pre-trigger: quarter (per-batch) input loads across 4 engines so G b0
    starts as soon as its slice lands; partial Gsum shears (s01 written at
    b1-done, s23 at b3-done, direct from PSUM with dup) -> 4-acc part
    matmul in f32r (1-pass PE) -> arin.  Trigger ~5us earlier.
  - post-CC critical chain shortened: Hankel(e4) is pre-built DURING the
    collective window (e3a bounce); post-CC only the 384B top-5 MASK is
    bounced (mask3) and h1 = Hankel(e4) * Hankel(mask) on DVE.  max8 /
    is_ge read the rank-sum straight from PSUM (no SBUF copy).
  - unshear stage batched per PAIR (m3a writes/mrev reads cover 2 batches
    each: 4+4 DMAs instead of 8+8), mrev reads moved to gpsimd (idle),
    descgen ~0.62-0.85us per dma_start was the stage bottleneck.
  - final matmuls compute outT (he-chunk rows x 192 time cols): 1536 PE
    cycles/batch instead of 2048; osum scale-copies split across
    vector/scalar/gpsimd; single out write per pair (4-level AP).
  - no post-CC dummy matmuls (PE is duty-cycle limited to ~1.2GHz
    sustained; they only stole slots).

Measured v1 (8-core axon pod): exec 113-120us (core 0 = first-launched,
pays full launch skew in the collective rendezvous).  rel_err 5.763e-03.
"""

import numpy as np

B, L, H, E = 32, 192, 8, 64
HE = H * E
N_CORES = 8
PER = B // N_CORES
L2 = 2 * L
BSTR = L * 576            # per-batch stride in shear scratch arrays
SREG = L * 576 + 640      # one shear region (s01 / s23)

USE_ALLGATHER = False
USE_F32R = True           # 1-pass PE mode for the fp32 part-sum matmul
N_WARM = 8                # PE pstate warm-up matmuls

_compiled = {}


def _build():
    import concourse.bacc as bacc
    import concourse.mybir as mybir
    from concourse.bass_types import AP
    from concourse.tile import TileContext, add_dep_helper

    f32 = mybir.dt.float32
    f32r = mybir.dt.float32r if USE_F32R else mybir.dt.float32
    bf = mybir.dt.bfloat16
    Exp = mybir.ActivationFunctionType.Exp
    CopyF = mybir.ActivationFunctionType.Copy
    Alu = mybir.AluOpType
    Ax = mybir.AxisListType

    nc = bacc.Bacc("TRN2", target_bir_lowering=False, debug=False,
                   num_devices=N_CORES, num_swdge_queues=2)

    # ---- dram I/O (host-packed layouts, see kernel()) ----
    kt_d = nc.dram_tensor("kt", [128, PER * 4 * L], bf, kind="ExternalInput")
    qt_d = nc.dram_tensor("qt", [128, PER * 4 * L], bf, kind="ExternalInput")
    kf_d = nc.dram_tensor("kf", [128, PER * 4 * L], bf, kind="ExternalInput")
    vt_d = nc.dram_tensor("vt", [128, PER * 4 * L], bf, kind="ExternalInput")
    qr0_d = nc.dram_tensor("qr0", [128, PER * HE], bf, kind="ExternalInput")
    qr1_d = nc.dram_tensor("qr1", [64, PER * HE], bf, kind="ExternalInput")
    outT_d = nc.dram_tensor("outT", [HE, PER * L], bf, kind="ExternalOutput")

    # ---- dram scratch ----
    gs3 = nc.dram_tensor("gs3", [4 * SREG], f32r)              # per-batch G shear
    g3a = nc.dram_tensor("g3a", [PER * BSTR + 640], f32r)      # per-batch G bounce
    w3 = nc.dram_tensor("w3", [PER * BSTR + 640], bf)          # W shear bounce
    m3a = nc.dram_tensor("m3a", [PER * BSTR + 640], bf)        # M unshear bounce
    e3a = nc.dram_tensor("e3a", [PER * 576 + 640], bf)         # tripled e4 (Hankel)
    # arin carries gsum DUPLICATED (1, 2L): the AllReduce output then serves
    # directly as the doubled buffer for Hankel-shifted mask reads -- no
    # post-collective DRAM bounce at all.
    arin = nc.dram_tensor("arin", [1, 3 * L], f32)
    arout = nc.dram_tensor("arout", [1, 3 * L], f32, addr_space="Shared")

    PCH = [(0, 128), (128, 64)]
    HCH = [0, 128, 256, 384]

    with TileContext(nc) as tc:
        with tc.tile_pool(name="sb", bufs=1) as sb, \
             tc.tile_pool(name="ps", bufs=1, space="PSUM") as ps:

            # ================= constants (off critical path) =================
            warm_t = sb.tile([128, 256], bf, tag="warm_t")
            nc.vector.memset(warm_t[:, :], 0.125)
            ones_t = sb.tile([128, 1], f32, tag="ones")
            nc.vector.memset(ones_t[:, :], 1.0)
            ones_r = sb.tile([128, 1], f32r, tag="ones_r")
            nc.gpsimd.dma_start(out=ones_r[:, :], in_=ones_t[:, :])
            oh_t = sb.tile([128, 16], f32, tag="oh")   # one-hot blocks: col 4b+b is 1
            nc.vector.memset(oh_t[:, :], 0.0)
            for b in range(PER):
                nc.vector.memset(oh_t[:, 4 * b + b: 4 * b + b + 1], 1.0)
            oh_r = sb.tile([128, 16], f32r, tag="oh_r")
            nc.gpsimd.dma_start(out=oh_r[:, :], in_=oh_t[:, :])

            # ============ input loads: per-batch quarters on 4 engines ======
            QL = 4 * L                       # one batch's columns
            kt_t = sb.tile([128, PER * 4 * L], bf, tag="kt_t")
            qt_t = sb.tile([128, PER * 4 * L], bf, tag="qt_t")
            vt_t = sb.tile([128, PER * 4 * L], bf, tag="vt_t")
            kf_t = sb.tile([128, PER * 4 * L], bf, tag="kf_t")
            qr0_t = sb.tile([128, PER * HE], bf, tag="qr0_t")
            qr1_t = sb.tile([64, PER * HE], bf, tag="qr1_t")
            kt_eng = [nc.sync, nc.scalar, nc.scalar, nc.sync]
            qt_eng = [nc.gpsimd, nc.sync, nc.gpsimd, nc.scalar]
            for b in range(PER):
                kt_eng[b].dma_start(out=kt_t[:, b * QL:(b + 1) * QL],
                                    in_=kt_d[:, b * QL:(b + 1) * QL])
                qt_eng[b].dma_start(out=qt_t[:, b * QL:(b + 1) * QL],
                                    in_=qt_d[:, b * QL:(b + 1) * QL])

            # ================= PE warm-up =================
            for w in range(N_WARM):
                wp = ps.tile([128, 256], f32, tag="warm_p", bufs=1)
                nc.tensor.matmul(wp[:, :], warm_t[:, 0:128], warm_t[:, :],
                                 start=True, stop=True)

            # ========= G matmuls + dup copies + per-batch shear bounce ======
            # The gcp dup copies ARE the duplicated per-batch G, so each
            # batch's shear region is written straight from gcp the moment
            # its copy lands (no partial-sum adds); the partition sum over
            # all 4 batches happens in the 8-acc part matmul after readback.
            gcp = {}   # (chunk) -> tile (mn, PER*2L) f32r, per-batch rows dup'd
            for ci, (m0, mn) in enumerate(PCH):
                gcp[m0] = sb.tile([mn, PER * L2], f32r, tag=f"gcp{m0}",
                                  name=f"gcp{m0}")
            gssh = {}  # (pair, m0) -> [mn, L] sheared partial Gsum
            gps = {}   # (b, m0) -> PSUM tile for the pair adds
            sdup = {}
            wr_eng = [nc.sync, nc.scalar]
            rd_eng = [nc.scalar, nc.sync]
            add_i = None
            for b in range(PER):
                for ci, (m0, mn) in enumerate(PCH):
                    gp = ps.tile([mn, L], f32, tag="mm", bufs=4)
                    gps[(b, m0)] = gp
                    for i in range(4):
                        nc.tensor.matmul(
                            gp[:, :],
                            kt_t[:, (b * 4 + i) * L + m0:(b * 4 + i) * L + m0 + mn],
                            qt_t[:, (b * 4 + i) * L:(b * 4 + i + 1) * L],
                            start=(i == 0), stop=(i == 3))
                    dst = gcp[m0][:, b * L2:(b + 1) * L2] \
                        .rearrange("p (r l) -> p r l", r=2)
                    src = gp[:, :].unsqueeze(1).broadcast_to((mn, 2, L))
                    if ci == 0:
                        cp_i = nc.vector.tensor_copy(dst, src)
                        if b == 2 and add_i is not None:
                            # keep the s01 adds ahead of later copies in the
                            # vector queue (the scheduler doesn't know the
                            # adds gate the collective trigger)
                            add_dep_helper(cp_i.ins, add_i.ins, sync=False,
                                           reason="s01 adds before b2 copy")
                    else:
                        nc.scalar.activation(dst, src, CopyF, bias=0.0, scale=1.0)
                if b % 2 == 1:   # pair complete -> dup'd partial sum on vector
                    pr = b // 2
                    for ci, (m0, mn) in enumerate(PCH):
                        t = sb.tile([mn, L2], f32r, tag=f"sd{pr}_{m0}",
                                    name=f"sd{pr}_{m0}")
                        add_i = nc.vector.tensor_add(
                            t[:, :].rearrange("p (r l) -> p r l", r=2),
                            gcp[m0][:, (b - 1) * L2:b * L2]
                                .rearrange("p (r l) -> p r l", r=2),
                            gps[(b, m0)][:, :].unsqueeze(1)
                                .broadcast_to((mn, 2, L)))
                        sdup[(pr, m0)] = t
                    for ci, (m0, mn) in enumerate(PCH):
                        wr_eng[ci].dma_start(
                            out=AP(tensor=gs3,
                                   offset=pr * SREG + 192 + m0 * 575,
                                   ap=[[575, mn], [1, L2]]),
                            in_=sdup[(pr, m0)][:, :])
                    for ci, (m0, mn) in enumerate(PCH):
                        t = sb.tile([mn, L], f32r, tag=f"gssh{pr}_{m0}",
                                    name=f"gssh{pr}_{m0}")
                        rd_eng[ci].dma_start(
                            out=t[:, :],
                            in_=AP(tensor=gs3,
                                   offset=pr * SREG + 192 + m0 * 576,
                                   ap=[[576, mn], [1, L]]))
                        gssh[(pr, m0)] = t

            # 4-acc partition-sum matmul -> arin (gsum written DUPLICATED)
            engs = [nc.sync, nc.scalar]
            partp = ps.tile([4, L], f32, tag="dsm", bufs=1)
            k = 0
            for pr in range(2):
                for (m0, mn) in PCH:
                    nc.tensor.matmul(partp[0:1, :], ones_r[:mn, :],
                                     gssh[(pr, m0)][:, :],
                                     start=(k == 0), stop=(k == 3))
                    k += 1
            # materialize the doubled gsum in SBUF (dup in the DVE copy)
            # and write arin with a plain 2D DMA: a broadcast-AP DRAM write
            # here corrupted the next DMA on the sync queue (the kf load).
            part_sb = sb.tile([1, 3 * L], f32, tag="part")
            nc.vector.tensor_copy(
                part_sb[:, :].rearrange("p (r l) -> p r l", r=3),
                partp[0:1, :].unsqueeze(1).broadcast_to((1, 3, L)))
            arin_i = nc.sync.dma_start(out=arin[:, :], in_=part_sb[:, :])


            # ================= collective =================
            nc.gpsimd.collective_compute(
                "AllReduce", Alu.add,
                replica_groups=[list(range(N_CORES))],
                ins=[arin[:, :]], outs=[arout[:, :]])

            # ============ CC bring-up window: W loads + g3a bounce + W phase
            # + e4 + Hankel(e4) pre-build.  Explicit deps on arin keep the
            # scheduler from hoisting these transfers into the critical
            # pre-trigger window.
            flight = []
            flight.append(nc.sync.dma_start(out=kf_t[:, :], in_=kf_d[:, :]))
            flight.append(nc.scalar.dma_start(out=vt_t[:, :], in_=vt_d[:, :]))
            flight.append(nc.sync.dma_start(out=qr0_t[:, :], in_=qr0_d[:, :]))
            flight.append(nc.scalar.dma_start(out=qr1_t[:, :], in_=qr1_d[:, :]))
            for ci, (m0, mn) in enumerate(PCH):
                flight.append(engs[ci].dma_start(
                    out=AP(tensor=g3a, offset=192 + m0 * 575,
                           ap=[[575, mn], [BSTR, PER], [1, L2]]),
                    in_=gcp[m0][:, :].rearrange("p (b l) -> p b l", b=PER)))
            gsh = {}
            for ci, (m0, mn) in enumerate(PCH):
                t = sb.tile([mn, PER * L], f32r, tag=f"gsh{m0}", name=f"gsh{m0}")
                engs[ci].dma_start(
                    out=t[:, :].rearrange("p (b l) -> p b l", b=PER),
                    in_=AP(tensor=g3a, offset=192 + m0 * 576,
                           ap=[[576, mn], [BSTR, PER], [1, L]]))
                gsh[m0] = t
            for i in flight:
                add_dep_helper(i.ins, arin_i.ins, sync=True,
                               reason="keep pre-trigger DMA window clear")

            # W matmuls + dup copies
            wcp = {}
            for (m0, mn) in PCH:
                wcp[m0] = sb.tile([mn, PER * L2], bf, tag=f"wcp{m0}", name=f"wcp{m0}")
            for b in range(PER):
                for ci, (m0, mn) in enumerate(PCH):
                    wp = ps.tile([mn, L], f32, tag="mm", bufs=4)
                    for i in range(4):
                        nc.tensor.matmul(
                            wp[:, :],
                            kf_t[:, (b * 4 + i) * L + m0:(b * 4 + i) * L + m0 + mn],
                            vt_t[:, (b * 4 + i) * L:(b * 4 + i + 1) * L],
                            start=(i == 0), stop=(i == 3))
                    dst = wcp[m0][:, b * L2:(b + 1) * L2] \
                        .rearrange("p (r l) -> p r l", r=2)
                    src = wp[:, :].unsqueeze(1).broadcast_to((mn, 2, L))
                    if ci == 0:
                        nc.vector.tensor_copy(dst, src)
                    else:
                        nc.scalar.activation(dst, src, CopyF, bias=0.0, scale=1.0)
            for ci, (m0, mn) in enumerate(PCH):
                engs[ci].dma_start(
                    out=AP(tensor=w3, offset=192 + m0 * 575,
                           ap=[[575, mn], [BSTR, PER], [1, L2]]),
                    in_=wcp[m0][:, :].rearrange("p (b l) -> p b l", b=PER))
            wsh = {}
            for ci, (m0, mn) in enumerate(PCH):
                t = sb.tile([mn, PER * L], bf, tag=f"wsh{m0}", name=f"wsh{m0}")
                engs[ci].dma_start(
                    out=t[:, :].rearrange("p (b l) -> p b l", b=PER),
                    in_=AP(tensor=w3, offset=192 + m0 * 576,
                           ap=[[576, mn], [BSTR, PER], [1, L]]))
                wsh[m0] = t

            # per-batch D -> e4 (f32 for the z-path, bf16 for the Hankel)
            d4p = ps.tile([4, L], f32, tag="dsm", bufs=1)
            nmm = 2 * PER
            k = 0
            for b in range(PER):
                for (m0, mn) in PCH:
                    nc.tensor.matmul(d4p[:, :], oh_r[:mn, 4 * b:4 * b + 4],
                                     gsh[m0][:, b * L:(b + 1) * L],
                                     start=(k == 0), stop=(k == nmm - 1))
                    k += 1
            e4b = sb.tile([PER, L], bf, tag="e4b")
            nc.scalar.activation(e4b[:, :], d4p[:, :], Exp, bias=0.0,
                                 scale=1.0 / HE)

            # Hankel(e4) pre-build: tripled e4 to DRAM, read back shifted
            # per partition.  All inside the CC window.
            nc.gpsimd.dma_start(
                out=AP(tensor=e3a, offset=0, ap=[[576, PER], [192, 3], [1, L]]),
                in_=e4b[:, :].unsqueeze(1).broadcast_to((PER, 3, L)))
            h1e = {}
            h1e_last = None
            for p0 in (0, 2):
                for ci, (m0, mn) in enumerate(PCH):
                    t = sb.tile([mn, 2 * L], bf, tag=f"h1e_{p0}_{m0}",
                                name=f"h1e_{p0}_{m0}")
                    h1e_last = nc.gpsimd.dma_start(
                        out=t[:, :].rearrange("p (b l) -> p b l", b=2),
                        in_=AP(tensor=e3a, offset=1 + m0 + 576 * p0,
                               ap=[[1, mn], [576, 2], [1, L]]))
                    h1e[(p0, m0)] = t

            # ================= post-collective =================
            # arout holds the TRIPLED global gsum; ONE gpsimd (SWDGE) DMA
            # reads both Hankel-shifted chunks -- HWDGE packets on the
            # Shared collective region are ~700ns each vs ~50ns on SWDGE.
            # Every Hankel row is a rotation of the full 192-vector, so a
            # per-partition MAX8 gives the global top-5 threshold.
            gHall = sb.tile([128, 2 * L], f32, tag="gHall")
            gh_i = nc.gpsimd.dma_start(
                out=gHall[:, :].rearrange("p (j l) -> p j l", j=2),
                in_=AP(tensor=arout, offset=1, ap=[[1, 128], [128, 2], [1, L]]))
            add_dep_helper(gh_i.ins, h1e_last.ins, sync=False,
                           reason="gH after window h1e reads")
            mHb = {}
            for ci, (m0, mn) in enumerate(PCH):
                sl = gHall[0:mn, ci * L:(ci + 1) * L]
                mx_t = sb.tile([mn, 8], f32, tag=f"mxH{m0}", name=f"mxH{m0}")
                nc.vector.max(out=mx_t[:, :], in_=sl)
                mb = sb.tile([mn, L], bf, tag=f"mHb{m0}", name=f"mHb{m0}")
                nc.vector.tensor_scalar(out=mb[:, :], in0=sl,
                                        scalar1=mx_t[:, 4:5], scalar2=None,
                                        op0=Alu.is_ge)
                mHb[m0] = mb
            h1t = {}
            for p0 in (0, 2):
                for ci, (m0, mn) in enumerate(PCH):
                    t = sb.tile([mn, 2 * L], bf, tag=f"h1_{p0}_{m0}",
                                name=f"h1_{p0}_{m0}")
                    nc.vector.tensor_tensor(
                        out=t[:, :].rearrange("p (r l) -> p r l", r=2),
                        in0=h1e[(p0, m0)][:, :].rearrange("p (r l) -> p r l", r=2),
                        in1=mHb[m0][:, :].unsqueeze(1).broadcast_to((mn, 2, L)),
                        op=Alu.mult)
                    h1t[(p0, m0)] = t

            # ====== T1 = HankelC^T @ shear(W); per-PAIR unshear bounce ======
            tdup = {}
            mrev = {}
            t1_last = None
            cast_last = None
            MSTR = BSTR
            for p0 in (0, 2):
                for ci, (m0, mn) in enumerate(PCH):
                    tdup[(p0, m0)] = sb.tile([mn, 2 * L2], bf,
                                             tag=f"tdup{p0}_{m0}",
                                             name=f"tdup{p0}_{m0}")
                for b in (p0, p0 + 1):
                    for ci, (m0, mn) in enumerate(PCH):
                        tp = ps.tile([mn, L], f32, tag="mm", bufs=4)
                        for i, (u0, un) in enumerate(PCH):
                            t1_last = nc.tensor.matmul(
                                tp[:, :],
                                h1t[(p0, u0)][:, (b - p0) * L + m0:
                                              (b - p0) * L + m0 + mn],
                                wsh[u0][:, b * L:(b + 1) * L],
                                start=(i == 0), stop=(i == 1))
                        dst = tdup[(p0, m0)][:, (b - p0) * L2:(b - p0 + 1) * L2] \
                            .rearrange("p (r l) -> p r l", r=2)
                        src = tp[:, :].unsqueeze(1).broadcast_to((mn, 2, L))
                        if ci == 0:
                            cast_last = nc.vector.tensor_copy(dst, src)
                        else:
                            nc.scalar.activation(dst, src, CopyF, bias=0.0,
                                                 scale=1.0)
                # per-pair unshear writes (sync c0 / scalar c1), reads on the
                # OTHER engine so a read issues right when its write's
                # semaphore fires instead of queuing behind it
                wr_eng = [nc.sync, nc.scalar]
                rd_eng = [nc.scalar, nc.sync]
                for ci, (m0, mn) in enumerate(PCH):
                    wr_eng[ci].dma_start(
                        out=AP(tensor=m3a, offset=p0 * MSTR + 191 + m0 * 575,
                               ap=[[575, mn], [MSTR, 2], [1, L2]]),
                        in_=tdup[(p0, m0)][:, :]
                            .rearrange("p (j q) -> p j q", j=2))
                for ci, (m0, mn) in enumerate(PCH):
                    t = sb.tile([mn, 2 * L], bf, tag=f"mrev{p0}_{m0}",
                                name=f"mrev{p0}_{m0}")
                    rd_eng[ci].dma_start(
                        out=t[:, :].rearrange("p (j l) -> p j l", j=2),
                        in_=AP(tensor=m3a, offset=p0 * MSTR + 192 + m0 * 576,
                               ap=[[576, mn], [MSTR, 2], [1, L]]))
                    mrev[(p0, m0)] = t

            # 1/Z per batch: every row of h1t already contains the masked
            # weights over a full period, so a free-axis reduce of one
            # L-block gives Z_b replicated down all partitions -- no
            # cross-partition transpose/matmul chain needed at all.
            zrb = {}
            for b in range(PER):
                p0 = (b // 2) * 2
                zs = sb.tile([128, 1], f32, tag=f"zs{b}", name=f"zs{b}")
                nc.vector.tensor_reduce(
                    out=zs[:, :],
                    in_=h1t[(p0, 0)][:, (b - p0) * L:(b - p0 + 1) * L],
                    axis=Ax.X, op=Alu.add)
                zr_t = sb.tile([128, 1], f32, tag=f"zrb{b}", name=f"zrb{b}")
                nc.vector.reciprocal(zr_t[:, :], zs[:, :])
                zrb[b] = zr_t


            # ====== finals: outT[he, b*L+l] = sum_g qr[g,he] mrev[g,l] ======
            osum = {}
            for p0 in (0, 2):
                osum[p0] = sb.tile([128, 4 * L2], bf, tag=f"osum{p0}",
                                   name=f"osum{p0}")
            qrt = {0: qr0_t, 128: qr1_t}
            # osum engine split: h-chunks 0,1 -> vector, 2 -> scalar, 3 -> gpsimd
            for b in range(PER):
                p0 = (b // 2) * 2
                for hi, h0 in enumerate(HCH):
                    op_ = ps.tile([128, L], f32, tag="op", bufs=2)
                    for i, (i0, in_n) in enumerate(PCH):
                        nc.tensor.matmul(
                            op_[:, :],
                            qrt[i0][:, b * HE + h0:b * HE + h0 + 128],
                            mrev[(p0, i0)][:, (b - p0) * L:(b - p0 + 1) * L],
                            start=(i == 0), stop=(i == 1))
                    dst = osum[p0][:, hi * L2 + (b - p0) * L:
                                   hi * L2 + (b - p0 + 1) * L]
                    if hi < 2:
                        nc.vector.tensor_scalar(out=dst, in0=op_[:, :],
                                                scalar1=zrb[b][:, 0:1],
                                                scalar2=None, op0=Alu.mult)
                    else:
                        nc.scalar.activation(dst, op_[:, :], CopyF, bias=0.0,
                                             scale=zrb[b][:, 0:1])
                if b % 2 == 1:   # pair complete -> single 4-level output write
                    nc.sync.dma_start(
                        out=AP(tensor=outT_d, offset=p0 * L,
                               ap=[[PER * L, 128], [128 * PER * L, 4],
                                   [L, 2], [1, L]]),
                        in_=osum[p0][:, :]
                            .rearrange("p (h j l) -> p h j l", h=4, j=2))


    nc.finalize()
    return nc


def _get_nc():
    if "nc" not in _compiled:
        _compiled["nc"] = _build()
    return _compiled["nc"]


def kernel(queries, keys, values, adj, attn_mask):
    import ml_dtypes
    from concourse.bass_utils import run_bass_kernel_spmd

    bf16 = ml_dtypes.bfloat16
    queries = np.ascontiguousarray(np.asarray(queries, dtype=np.float32))
    keys = np.ascontiguousarray(np.asarray(keys, dtype=np.float32))
    values = np.ascontiguousarray(np.asarray(values, dtype=np.float32))

    def pack_proper(x):   # (PER,L,H,E) -> (128, PER*4*L): [p,(b*4+i)*L+s] = X[b,s,128i+p]
        t = x.reshape(PER, L, HE).transpose(0, 2, 1)
        t = t.reshape(PER, 4, 128, L).transpose(2, 0, 1, 3)
        return np.ascontiguousarray(t.reshape(128, PER * 4 * L)).astype(bf16)

    def pack_view(x):     # torch-style .view(HE, L) layout
        t = x.reshape(PER, HE, L)
        t = t.reshape(PER, 4, 128, L).transpose(2, 0, 1, 3)
        return np.ascontiguousarray(t.reshape(128, PER * 4 * L)).astype(bf16)

    def pack_qr(x):       # reversed (L,HE) per batch, split into row chunks
        t = x.reshape(PER, HE, L).transpose(0, 2, 1)[:, ::-1, :]
        a = np.ascontiguousarray(t[:, 0:128, :].transpose(1, 0, 2)
                                 .reshape(128, PER * HE)).astype(bf16)
        b = np.ascontiguousarray(t[:, 128:192, :].transpose(1, 0, 2)
                                 .reshape(64, PER * HE)).astype(bf16)
        return a, b

    nc = _get_nc()
    in_maps = []
    for c in range(N_CORES):
        sl = slice(c * PER, (c + 1) * PER)
        q, k, v = queries[sl], keys[sl], values[sl]
        qr0, qr1 = pack_qr(q)
        in_maps.append({
            "kt": pack_proper(k),
            "qt": pack_proper(q),
            "kf": pack_view(k),
            "vt": pack_proper(v),
            "qr0": qr0,
            "qr1": qr1,
        })

    res = run_bass_kernel_spmd(nc, in_maps, list(range(N_CORES)),
                               **_compiled.get("run_kwargs", {}))
    _compiled["last_result"] = res
    outs = [np.asarray(res.results[c]["outT"]).astype(np.float32)
            .reshape(HE, PER, L).transpose(1, 2, 0)
            .reshape(PER, L, H, E) for c in range(N_CORES)]
    return np.concatenate(outs, axis=0)


# revision 43
# speedup vs baseline: 1.1377x; 1.1377x over previous
"""Trainium2 Bass kernel for nn_AutoCorrelation_spa_tem.

Shards batch B=32 across 8 NeuronCores (4 batches/core, pure data parallel).

Algorithm (collapsed form of the reference):
  G_b   = keys[b](L,HE) @ queries[b](L,HE)^T            (192x192)
  D_raw[b,tau] = sum_s G_b[s,(s+tau)%L]                 (diag sums via shear)
  gsum  = AllGather_b(sum_b D_raw) + local sum -> top-5 mask via max8
  c_b   = mask * softmax(D_raw[b]/HE over selected)
  W_b   = keys[b].reshape(HE,L)^T @ values_proper(HE,L) (192x192)
  M_b   = sum_d c_b[d] * Shift_d(W_b)   [2D circular diagonal shift]
        = unshear(HankelC^T @ shear(W_b))   (all positive-stride DMAs)
  out[b] = (Qtilde_b @ M_b)^T  computed TRANSPOSED: outT = qr^T-chunks @ mrev
           (host transposes back; 192-col matmuls instead of 512-col)

v2 perf structure (vs v1 ~113-120us):
  - pre-trigger: quarter (per-batch) input loads across 4 engines so G b0
    starts as soon as its slice lands; partial Gsum shears (s01 written at
    b1-done, s23 at b3-done, direct from PSUM with dup) -> 4-acc part
    matmul in f32r (1-pass PE) -> arin.  Trigger ~5us earlier.
  - post-CC critical chain shortened: Hankel(e4) is pre-built DURING the
    collective window (e3a bounce); post-CC only the 384B top-5 MASK is
    bounced (mask3) and h1 = Hankel(e4) * Hankel(mask) on DVE.  max8 /
    is_ge read the rank-sum straight from PSUM (no SBUF copy).
  - unshear stage batched per PAIR (m3a writes/mrev reads cover 2 batches
    each: 4+4 DMAs instead of 8+8), mrev reads moved to gpsimd (idle),
    descgen ~0.62-0.85us per dma_start was the stage bottleneck.
  - final matmuls compute outT (he-chunk rows x 192 time cols): 1536 PE
    cycles/batch instead of 2048; osum scale-copies split across
    vector/scalar/gpsimd; single out write per pair (4-level AP).
  - no post-CC dummy matmuls (PE is duty-cycle limited to ~1.2GHz
    sustained; they only stole slots).

Measured v1 (8-core axon pod): exec 113-120us (core 0 = first-launched,
pays full launch skew in the collective rendezvous).  rel_err 5.763e-03.
"""

import numpy as np

B, L, H, E = 32, 192, 8, 64
HE = H * E
N_CORES = 8
PER = B // N_CORES
L2 = 2 * L
BSTR = L * 576            # per-batch stride in shear scratch arrays
SREG = L * 576 + 640      # one shear region (s01 / s23)

USE_ALLGATHER = False
USE_F32R = True           # 1-pass PE mode for the fp32 part-sum matmul
N_WARM = 8                # PE pstate warm-up matmuls

_compiled = {}


def _build():
    import concourse.bacc as bacc
    import concourse.mybir as mybir
    from concourse.bass_types import AP
    from concourse.tile import TileContext, add_dep_helper

    f32 = mybir.dt.float32
    f32r = mybir.dt.float32r if USE_F32R else mybir.dt.float32
    bf = mybir.dt.bfloat16
    Exp = mybir.ActivationFunctionType.Exp
    CopyF = mybir.ActivationFunctionType.Copy
    Alu = mybir.AluOpType
    Ax = mybir.AxisListType

    nc = bacc.Bacc("TRN2", target_bir_lowering=False, debug=False,
                   num_devices=N_CORES, num_swdge_queues=2)

    # ---- dram I/O (host-packed layouts, see kernel()) ----
    kt_d = nc.dram_tensor("kt", [128, PER * 4 * L], bf, kind="ExternalInput")
    qt_d = nc.dram_tensor("qt", [128, PER * 4 * L], bf, kind="ExternalInput")
    kf_d = nc.dram_tensor("kf", [128, PER * 4 * L], bf, kind="ExternalInput")
    vt_d = nc.dram_tensor("vt", [128, PER * 4 * L], bf, kind="ExternalInput")
    qr0_d = nc.dram_tensor("qr0", [128, PER * HE], bf, kind="ExternalInput")
    qr1_d = nc.dram_tensor("qr1", [64, PER * HE], bf, kind="ExternalInput")
    outT_d = nc.dram_tensor("outT", [HE, PER * L], bf, kind="ExternalOutput")

    # ---- dram scratch ----
    gs3 = nc.dram_tensor("gs3", [4 * SREG], f32r)              # per-batch G shear
    g3a = nc.dram_tensor("g3a", [PER * BSTR + 640], f32r)      # per-batch G bounce
    w3 = nc.dram_tensor("w3", [PER * BSTR + 640], bf)          # W shear bounce
    m3a = nc.dram_tensor("m3a", [PER * BSTR + 640], bf)        # M unshear bounce
    e3a = nc.dram_tensor("e3a", [PER * 576 + 640], bf)         # tripled e4 (Hankel)
    # arin carries gsum DUPLICATED (1, 2L): the AllReduce output then serves
    # directly as the doubled buffer for Hankel-shifted mask reads -- no
    # post-collective DRAM bounce at all.
    arin = nc.dram_tensor("arin", [1, 2 * L], f32)
    arout = nc.dram_tensor("arout", [1, 2 * L], f32, addr_space="Shared")

    PCH = [(0, 128), (128, 64)]
    HCH = [0, 128, 256, 384]

    with TileContext(nc) as tc:
        with tc.tile_pool(name="sb", bufs=1) as sb, \
             tc.tile_pool(name="ps", bufs=1, space="PSUM") as ps:

            # ================= constants (off critical path) =================
            warm_t = sb.tile([128, 256], bf, tag="warm_t")
            nc.vector.memset(warm_t[:, :], 0.125)
            ones_t = sb.tile([128, 1], f32, tag="ones")
            nc.vector.memset(ones_t[:, :], 1.0)
            ones_r = sb.tile([128, 1], f32r, tag="ones_r")
            nc.gpsimd.dma_start(out=ones_r[:, :], in_=ones_t[:, :])
            oh_t = sb.tile([128, 16], f32, tag="oh")   # one-hot blocks: col 4b+b is 1
            nc.vector.memset(oh_t[:, :], 0.0)
            for b in range(PER):
                nc.vector.memset(oh_t[:, 4 * b + b: 4 * b + b + 1], 1.0)
            oh_r = sb.tile([128, 16], f32r, tag="oh_r")
            nc.gpsimd.dma_start(out=oh_r[:, :], in_=oh_t[:, :])

            # ============ input loads: per-batch quarters on 4 engines ======
            QL = 4 * L                       # one batch's columns
            kt_t = sb.tile([128, PER * 4 * L], bf, tag="kt_t")
            qt_t = sb.tile([128, PER * 4 * L], bf, tag="qt_t")
            vt_t = sb.tile([128, PER * 4 * L], bf, tag="vt_t")
            kf_t = sb.tile([128, PER * 4 * L], bf, tag="kf_t")
            qr0_t = sb.tile([128, PER * HE], bf, tag="qr0_t")
            qr1_t = sb.tile([64, PER * HE], bf, tag="qr1_t")
            kt_eng = [nc.sync, nc.scalar, nc.scalar, nc.sync]
            qt_eng = [nc.gpsimd, nc.sync, nc.gpsimd, nc.scalar]
            for b in range(PER):
                kt_eng[b].dma_start(out=kt_t[:, b * QL:(b + 1) * QL],
                                    in_=kt_d[:, b * QL:(b + 1) * QL])
                qt_eng[b].dma_start(out=qt_t[:, b * QL:(b + 1) * QL],
                                    in_=qt_d[:, b * QL:(b + 1) * QL])

            # ================= PE warm-up =================
            for w in range(N_WARM):
                wp = ps.tile([128, 256], f32, tag="warm_p", bufs=1)
                nc.tensor.matmul(wp[:, :], warm_t[:, 0:128], warm_t[:, :],
                                 start=True, stop=True)

            # ========= G matmuls + dup copies + per-batch shear bounce ======
            # The gcp dup copies ARE the duplicated per-batch G, so each
            # batch's shear region is written straight from gcp the moment
            # its copy lands (no partial-sum adds); the partition sum over
            # all 4 batches happens in the 8-acc part matmul after readback.
            gcp = {}   # (chunk) -> tile (mn, PER*2L) f32r, per-batch rows dup'd
            for ci, (m0, mn) in enumerate(PCH):
                gcp[m0] = sb.tile([mn, PER * L2], f32r, tag=f"gcp{m0}",
                                  name=f"gcp{m0}")
            gssh = {}  # (pair, m0) -> [mn, L] sheared partial Gsum
            gps = {}   # (b, m0) -> PSUM tile for the pair adds
            sdup = {}
            wr_eng = [nc.sync, nc.scalar]
            rd_eng = [nc.scalar, nc.sync]
            add_i = None
            for b in range(PER):
                for ci, (m0, mn) in enumerate(PCH):
                    gp = ps.tile([mn, L], f32, tag="mm", bufs=4)
                    gps[(b, m0)] = gp
                    for i in range(4):
                        nc.tensor.matmul(
                            gp[:, :],
                            kt_t[:, (b * 4 + i) * L + m0:(b * 4 + i) * L + m0 + mn],
                            qt_t[:, (b * 4 + i) * L:(b * 4 + i + 1) * L],
                            start=(i == 0), stop=(i == 3))
                    dst = gcp[m0][:, b * L2:(b + 1) * L2] \
                        .rearrange("p (r l) -> p r l", r=2)
                    src = gp[:, :].unsqueeze(1).broadcast_to((mn, 2, L))
                    if ci == 0:
                        cp_i = nc.vector.tensor_copy(dst, src)
                        if b == 2 and add_i is not None:
                            # keep the s01 adds ahead of later copies in the
                            # vector queue (the scheduler doesn't know the
                            # adds gate the collective trigger)
                            add_dep_helper(cp_i.ins, add_i.ins, sync=False,
                                           reason="s01 adds before b2 copy")
                    else:
                        nc.scalar.activation(dst, src, CopyF, bias=0.0, scale=1.0)
                if b % 2 == 1:   # pair complete -> dup'd partial sum on vector
                    pr = b // 2
                    for ci, (m0, mn) in enumerate(PCH):
                        t = sb.tile([mn, L2], f32r, tag=f"sd{pr}_{m0}",
                                    name=f"sd{pr}_{m0}")
                        add_i = nc.vector.tensor_add(
                            t[:, :].rearrange("p (r l) -> p r l", r=2),
                            gcp[m0][:, (b - 1) * L2:b * L2]
                                .rearrange("p (r l) -> p r l", r=2),
                            gps[(b, m0)][:, :].unsqueeze(1)
                                .broadcast_to((mn, 2, L)))
                        sdup[(pr, m0)] = t
                    for ci, (m0, mn) in enumerate(PCH):
                        wr_eng[ci].dma_start(
                            out=AP(tensor=gs3,
                                   offset=pr * SREG + 192 + m0 * 575,
                                   ap=[[575, mn], [1, L2]]),
                            in_=sdup[(pr, m0)][:, :])
                    for ci, (m0, mn) in enumerate(PCH):
                        t = sb.tile([mn, L], f32r, tag=f"gssh{pr}_{m0}",
                                    name=f"gssh{pr}_{m0}")
                        rd_eng[ci].dma_start(
                            out=t[:, :],
                            in_=AP(tensor=gs3,
                                   offset=pr * SREG + 192 + m0 * 576,
                                   ap=[[576, mn], [1, L]]))
                        gssh[(pr, m0)] = t

            # 4-acc partition-sum matmul -> arin (gsum written DUPLICATED)
            engs = [nc.sync, nc.scalar]
            partp = ps.tile([4, L], f32, tag="dsm", bufs=1)
            k = 0
            for pr in range(2):
                for (m0, mn) in PCH:
                    nc.tensor.matmul(partp[0:1, :], ones_r[:mn, :],
                                     gssh[(pr, m0)][:, :],
                                     start=(k == 0), stop=(k == 3))
                    k += 1
            # materialize the doubled gsum in SBUF (dup in the DVE copy)
            # and write arin with a plain 2D DMA: a broadcast-AP DRAM write
            # here corrupted the next DMA on the sync queue (the kf load).
            part_sb = sb.tile([1, 2 * L], f32, tag="part")
            nc.vector.tensor_copy(
                part_sb[:, :].rearrange("p (r l) -> p r l", r=2),
                partp[0:1, :].unsqueeze(1).broadcast_to((1, 2, L)))
            arin_i = nc.sync.dma_start(out=arin[:, :], in_=part_sb[:, :])


            # ================= collective =================
            nc.gpsimd.collective_compute(
                "AllReduce", Alu.add,
                replica_groups=[list(range(N_CORES))],
                ins=[arin[:, :]], outs=[arout[:, :]])

            # ============ CC bring-up window: W loads + g3a bounce + W phase
            # + e4 + Hankel(e4) pre-build.  Explicit deps on arin keep the
            # scheduler from hoisting these transfers into the critical
            # pre-trigger window.
            flight = []
            flight.append(nc.sync.dma_start(out=kf_t[:, :], in_=kf_d[:, :]))
            flight.append(nc.scalar.dma_start(out=vt_t[:, :], in_=vt_d[:, :]))
            flight.append(nc.sync.dma_start(out=qr0_t[:, :], in_=qr0_d[:, :]))
            flight.append(nc.scalar.dma_start(out=qr1_t[:, :], in_=qr1_d[:, :]))
            for ci, (m0, mn) in enumerate(PCH):
                flight.append(engs[ci].dma_start(
                    out=AP(tensor=g3a, offset=192 + m0 * 575,
                           ap=[[575, mn], [BSTR, PER], [1, L2]]),
                    in_=gcp[m0][:, :].rearrange("p (b l) -> p b l", b=PER)))
            gsh = {}
            for ci, (m0, mn) in enumerate(PCH):
                t = sb.tile([mn, PER * L], f32r, tag=f"gsh{m0}", name=f"gsh{m0}")
                engs[ci].dma_start(
                    out=t[:, :].rearrange("p (b l) -> p b l", b=PER),
                    in_=AP(tensor=g3a, offset=192 + m0 * 576,
                           ap=[[576, mn], [BSTR, PER], [1, L]]))
                gsh[m0] = t
            for i in flight:
                add_dep_helper(i.ins, arin_i.ins, sync=True,
                               reason="keep pre-trigger DMA window clear")

            # W matmuls + dup copies
            wcp = {}
            for (m0, mn) in PCH:
                wcp[m0] = sb.tile([mn, PER * L2], bf, tag=f"wcp{m0}", name=f"wcp{m0}")
            for b in range(PER):
                for ci, (m0, mn) in enumerate(PCH):
                    wp = ps.tile([mn, L], f32, tag="mm", bufs=4)
                    for i in range(4):
                        nc.tensor.matmul(
                            wp[:, :],
                            kf_t[:, (b * 4 + i) * L + m0:(b * 4 + i) * L + m0 + mn],
                            vt_t[:, (b * 4 + i) * L:(b * 4 + i + 1) * L],
                            start=(i == 0), stop=(i == 3))
                    dst = wcp[m0][:, b * L2:(b + 1) * L2] \
                        .rearrange("p (r l) -> p r l", r=2)
                    src = wp[:, :].unsqueeze(1).broadcast_to((mn, 2, L))
                    if ci == 0:
                        nc.vector.tensor_copy(dst, src)
                    else:
                        nc.scalar.activation(dst, src, CopyF, bias=0.0, scale=1.0)
            for ci, (m0, mn) in enumerate(PCH):
                engs[ci].dma_start(
                    out=AP(tensor=w3, offset=192 + m0 * 575,
                           ap=[[575, mn], [BSTR, PER], [1, L2]]),
                    in_=wcp[m0][:, :].rearrange("p (b l) -> p b l", b=PER))
            wsh = {}
            for ci, (m0, mn) in enumerate(PCH):
                t = sb.tile([mn, PER * L], bf, tag=f"wsh{m0}", name=f"wsh{m0}")
                engs[ci].dma_start(
                    out=t[:, :].rearrange("p (b l) -> p b l", b=PER),
                    in_=AP(tensor=w3, offset=192 + m0 * 576,
                           ap=[[576, mn], [BSTR, PER], [1, L]]))
                wsh[m0] = t

            # per-batch D -> e4 (f32 for the z-path, bf16 for the Hankel)
            d4p = ps.tile([4, L], f32, tag="dsm", bufs=1)
            nmm = 2 * PER
            k = 0
            for b in range(PER):
                for (m0, mn) in PCH:
                    nc.tensor.matmul(d4p[:, :], oh_r[:mn, 4 * b:4 * b + 4],
                                     gsh[m0][:, b * L:(b + 1) * L],
                                     start=(k == 0), stop=(k == nmm - 1))
                    k += 1
            e4b = sb.tile([PER, L], bf, tag="e4b")
            nc.scalar.activation(e4b[:, :], d4p[:, :], Exp, bias=0.0,
                                 scale=1.0 / HE)

            # Hankel(e4) pre-build: tripled e4 to DRAM, read back shifted
            # per partition.  All inside the CC window.
            nc.gpsimd.dma_start(
                out=AP(tensor=e3a, offset=0, ap=[[576, PER], [192, 3], [1, L]]),
                in_=e4b[:, :].unsqueeze(1).broadcast_to((PER, 3, L)))
            h1e = {}
            h1e_last = None
            for p0 in (0, 2):
                for ci, (m0, mn) in enumerate(PCH):
                    t = sb.tile([mn, 2 * L], bf, tag=f"h1e_{p0}_{m0}",
                                name=f"h1e_{p0}_{m0}")
                    h1e_last = nc.gpsimd.dma_start(
                        out=t[:, :].rearrange("p (b l) -> p b l", b=2),
                        in_=AP(tensor=e3a, offset=1 + m0 + 576 * p0,
                               ap=[[1, mn], [576, 2], [1, L]]))
                    h1e[(p0, m0)] = t

            # ================= post-collective =================
            # arout holds the DOUBLED global gsum; read it in Hankel-
            # shifted form on sync+scalar in parallel (HWDGE packets on the
            # Shared region are ~700ns each; SWDGE packets are fast but its
            # completion semaphore is ~6us -- HWDGE wins for the chain).
            # Every Hankel row is a rotation of the full 192-vector, so a
            # per-partition MAX8 gives the global top-5 threshold.
            gH = {}
            gh_eng = [nc.sync, nc.scalar]
            mHb = {}
            for ci, (m0, mn) in enumerate(PCH):
                t = sb.tile([mn, L], f32, tag=f"gH{m0}", name=f"gH{m0}")
                gh_eng[ci].dma_start(
                    out=t[:, :],
                    in_=AP(tensor=arout, offset=1 + m0, ap=[[1, mn], [1, L]]))
                gH[m0] = t
            for ci, (m0, mn) in enumerate(PCH):
                mx_t = sb.tile([mn, 8], f32, tag=f"mxH{m0}", name=f"mxH{m0}")
                nc.vector.max(out=mx_t[:, :], in_=gH[m0][:, :])
                mb = sb.tile([mn, L], bf, tag=f"mHb{m0}", name=f"mHb{m0}")
                nc.vector.tensor_scalar(out=mb[:, :], in0=gH[m0][:, :],
                                        scalar1=mx_t[:, 4:5], scalar2=None,
                                        op0=Alu.is_ge)
                mHb[m0] = mb
            h1t = {}
            for p0 in (0, 2):
                for ci, (m0, mn) in enumerate(PCH):
                    t = sb.tile([mn, 2 * L], bf, tag=f"h1_{p0}_{m0}",
                                name=f"h1_{p0}_{m0}")
                    nc.vector.tensor_tensor(
                        out=t[:, :].rearrange("p (r l) -> p r l", r=2),
                        in0=h1e[(p0, m0)][:, :].rearrange("p (r l) -> p r l", r=2),
                        in1=mHb[m0][:, :].unsqueeze(1).broadcast_to((mn, 2, L)),
                        op=Alu.mult)
                    h1t[(p0, m0)] = t

            # ====== T1 = HankelC^T @ shear(W); per-PAIR unshear bounce ======
            tdup = {}
            mrev = {}
            t1_last = None
            cast_last = None
            MSTR = BSTR
            for p0 in (0, 2):
                for ci, (m0, mn) in enumerate(PCH):
                    tdup[(p0, m0)] = sb.tile([mn, 2 * L2], bf,
                                             tag=f"tdup{p0}_{m0}",
                                             name=f"tdup{p0}_{m0}")
                for b in (p0, p0 + 1):
                    for ci, (m0, mn) in enumerate(PCH):
                        tp = ps.tile([mn, L], f32, tag="mm", bufs=4)
                        for i, (u0, un) in enumerate(PCH):
                            t1_last = nc.tensor.matmul(
                                tp[:, :],
                                h1t[(p0, u0)][:, (b - p0) * L + m0:
                                              (b - p0) * L + m0 + mn],
                                wsh[u0][:, b * L:(b + 1) * L],
                                start=(i == 0), stop=(i == 1))
                        dst = tdup[(p0, m0)][:, (b - p0) * L2:(b - p0 + 1) * L2] \
                            .rearrange("p (r l) -> p r l", r=2)
                        src = tp[:, :].unsqueeze(1).broadcast_to((mn, 2, L))
                        if ci == 0:
                            cast_last = nc.vector.tensor_copy(dst, src)
                        else:
                            nc.scalar.activation(dst, src, CopyF, bias=0.0,
                                                 scale=1.0)
                # per-pair unshear writes (sync c0 / scalar c1), reads on the
                # OTHER engine so a read issues right when its write's
                # semaphore fires instead of queuing behind it
                wr_eng = [nc.sync, nc.scalar]
                rd_eng = [nc.scalar, nc.sync]
                for ci, (m0, mn) in enumerate(PCH):
                    wr_eng[ci].dma_start(
                        out=AP(tensor=m3a, offset=p0 * MSTR + 191 + m0 * 575,
                               ap=[[575, mn], [MSTR, 2], [1, L2]]),
                        in_=tdup[(p0, m0)][:, :]
                            .rearrange("p (j q) -> p j q", j=2))
                for ci, (m0, mn) in enumerate(PCH):
                    t = sb.tile([mn, 2 * L], bf, tag=f"mrev{p0}_{m0}",
                                name=f"mrev{p0}_{m0}")
                    rd_eng[ci].dma_start(
                        out=t[:, :].rearrange("p (j l) -> p j l", j=2),
                        in_=AP(tensor=m3a, offset=p0 * MSTR + 192 + m0 * 576,
                               ap=[[576, mn], [MSTR, 2], [1, L]]))
                    mrev[(p0, m0)] = t

            # 1/Z per batch: every row of h1t already contains the masked
            # weights over a full period, so a free-axis reduce of one
            # L-block gives Z_b replicated down all partitions -- no
            # cross-partition transpose/matmul chain needed at all.
            zrb = {}
            for b in range(PER):
                p0 = (b // 2) * 2
                zs = sb.tile([128, 1], f32, tag=f"zs{b}", name=f"zs{b}")
                nc.vector.tensor_reduce(
                    out=zs[:, :],
                    in_=h1t[(p0, 0)][:, (b - p0) * L:(b - p0 + 1) * L],
                    axis=Ax.X, op=Alu.add)
                zr_t = sb.tile([128, 1], f32, tag=f"zrb{b}", name=f"zrb{b}")
                nc.vector.reciprocal(zr_t[:, :], zs[:, :])
                zrb[b] = zr_t


            # ====== finals: outT[he, b*L+l] = sum_g qr[g,he] mrev[g,l] ======
            osum = {}
            for p0 in (0, 2):
                osum[p0] = sb.tile([128, 4 * L2], bf, tag=f"osum{p0}",
                                   name=f"osum{p0}")
            qrt = {0: qr0_t, 128: qr1_t}
            # osum engine split: h-chunks 0,1 -> vector, 2 -> scalar, 3 -> gpsimd
            for b in range(PER):
                p0 = (b // 2) * 2
                for hi, h0 in enumerate(HCH):
                    op_ = ps.tile([128, L], f32, tag="op", bufs=2)
                    for i, (i0, in_n) in enumerate(PCH):
                        nc.tensor.matmul(
                            op_[:, :],
                            qrt[i0][:, b * HE + h0:b * HE + h0 + 128],
                            mrev[(p0, i0)][:, (b - p0) * L:(b - p0 + 1) * L],
                            start=(i == 0), stop=(i == 1))
                    dst = osum[p0][:, hi * L2 + (b - p0) * L:
                                   hi * L2 + (b - p0 + 1) * L]
                    if hi < 2:
                        nc.vector.tensor_scalar(out=dst, in0=op_[:, :],
                                                scalar1=zrb[b][:, 0:1],
                                                scalar2=None, op0=Alu.mult)
                    else:
                        nc.scalar.activation(dst, op_[:, :], CopyF, bias=0.0,
                                             scale=zrb[b][:, 0:1])
                if b % 2 == 1:   # pair complete -> single 4-level output write
                    nc.sync.dma_start(
                        out=AP(tensor=outT_d, offset=p0 * L,
                               ap=[[PER * L, 128], [128 * PER * L, 4],
                                   [L, 2], [1, L]]),
                        in_=osum[p0][:, :]
                            .rearrange("p (h j l) -> p h j l", h=4, j=2))


    nc.finalize()
    return nc


def _get_nc():
    if "nc" not in _compiled:
        _compiled["nc"] = _build()
    return _compiled["nc"]


def kernel(queries, keys, values, adj, attn_mask):
    import ml_dtypes
    from concourse.bass_utils import run_bass_kernel_spmd

    bf16 = ml_dtypes.bfloat16
    queries = np.ascontiguousarray(np.asarray(queries, dtype=np.float32))
    keys = np.ascontiguousarray(np.asarray(keys, dtype=np.float32))
    values = np.ascontiguousarray(np.asarray(values, dtype=np.float32))

    def pack_proper(x):   # (PER,L,H,E) -> (128, PER*4*L): [p,(b*4+i)*L+s] = X[b,s,128i+p]
        t = x.reshape(PER, L, HE).transpose(0, 2, 1)
        t = t.reshape(PER, 4, 128, L).transpose(2, 0, 1, 3)
        return np.ascontiguousarray(t.reshape(128, PER * 4 * L)).astype(bf16)

    def pack_view(x):     # torch-style .view(HE, L) layout
        t = x.reshape(PER, HE, L)
        t = t.reshape(PER, 4, 128, L).transpose(2, 0, 1, 3)
        return np.ascontiguousarray(t.reshape(128, PER * 4 * L)).astype(bf16)

    def pack_qr(x):       # reversed (L,HE) per batch, split into row chunks
        t = x.reshape(PER, HE, L).transpose(0, 2, 1)[:, ::-1, :]
        a = np.ascontiguousarray(t[:, 0:128, :].transpose(1, 0, 2)
                                 .reshape(128, PER * HE)).astype(bf16)
        b = np.ascontiguousarray(t[:, 128:192, :].transpose(1, 0, 2)
                                 .reshape(64, PER * HE)).astype(bf16)
        return a, b

    nc = _get_nc()
    in_maps = []
    for c in range(N_CORES):
        sl = slice(c * PER, (c + 1) * PER)
        q, k, v = queries[sl], keys[sl], values[sl]
        qr0, qr1 = pack_qr(q)
        in_maps.append({
            "kt": pack_proper(k),
            "qt": pack_proper(q),
            "kf": pack_view(k),
            "vt": pack_proper(v),
            "qr0": qr0,
            "qr1": qr1,
        })

    res = run_bass_kernel_spmd(nc, in_maps, list(range(N_CORES)),
                               **_compiled.get("run_kwargs", {}))
    _compiled["last_result"] = res
    outs = [np.asarray(res.results[c]["outT"]).astype(np.float32)
            .reshape(HE, PER, L).transpose(1, 2, 0)
            .reshape(PER, L, H, E) for c in range(N_CORES)]
    return np.concatenate(outs, axis=0)
